# revision 4
# baseline (speedup 1.0000x reference)
"""Trainium2 Bass kernel for nn_DiagnosticRNN (LSTM B=2048,T=128,V=25,H=512
-> FC 100), 8-way batch-data-parallel across NeuronCores.

Strategy
--------
Data-parallel over batch: each of the 8 cores runs the full T=128 LSTM
recurrence on BS=256 batch rows with all weights replicated (per the
sharding hint). Everything is fused on-chip; the naive approach's
[B,T,4H] x-projection (2.1 GB of DRAM traffic) is never materialized.

Per-core per-timestep (all matmul operands bf16, fp32 PSUM accumulate),
scheduled to keep the PE array continuously busy (p-state stays at
2.4 GHz) and overlap all ACT/DVE/Pool work under the matmuls:

  X phase: 16 thin K=26 matmuls (V=25 + ones row folding b_ih+b_hh)
    compute the x-projection for all 16 gate m-tiles (permuted order:
    m = 4j+q, q in (i,f,g,o), j the h-chunk), 4-way row-tiled via
    tile_position. Emitted FIRST each step: it depends only on x, so it
    gives the PE deferred work while the previous step's h chunks finish.
  H phase, group-major: for each h-chunk group j (4 m-tiles = 2 PSUM
    banks), the 16 W_hh matmuls (k inner, ascending, so the
    last-produced h chunk k=3 is needed as late as possible). The
    group's elementwise chain is issued right after its last matmul:
    ScalarE sigmoid([i_j f_j]) (fused 512 cols), tanh(g_j), sigmoid(o_j)
    from PSUM, then VectorE ig = i*g, GpSimd fc = f*c, VectorE
    c = ig+fc, ScalarE tanh(c), VectorE h = o*tanh(c) (bf16, DVE 2x).
    So group j's activations overlap groups j+1..3's matmuls, and the
    whole chain for group 3 fits under the next step's X phase +
    deferred k<3 sweeps.
  h is double-buffered across steps (ping-pong) so the h write never
  WAR-stalls against the 16 cross-group reads of the previous value.
  t=0 is specialized: no h/c yet, so the hidden matmuls, f*c and the
  memsets are skipped entirely (gates = x-term only, c = i*g).
  FC epilogue: out[100, BS] = W_fc @ h (+b_fc via ACT Identity bias);
  host transposes to [BS, 100].

Host side packs/permutes/casts the weights and pre-transposes messages
into x_rep [128, T*BS] bf16 (4 replicas of [V+1, t, b] at partition
offsets 0/32/64/96). All numerics on device; bf16 operand rounding gives
~5e-3 scale-relative absmax vs the fp32 reference.
"""

import numpy as np
import ml_dtypes

import concourse.bacc as bacc
import concourse.mybir as mybir
import concourse.tile as tile
from concourse.bass_utils import run_bass_kernel_spmd

F32 = mybir.dt.float32
BF16 = mybir.dt.bfloat16
AF = mybir.ActivationFunctionType

B, T, V = 2048, 128, 25
H = 512
NCLS = 100
CORES = 8
BS = B // CORES          # 256 batch rows per core
KT = H // 128            # 4 k-tiles (h chunks)
MT = (4 * H) // 128      # 16 m-tiles
NB = 8                   # psum banks


def _gate_perm():
    """Permutation of the 4H gate dim: m-tile m=4j+q -> gate q, h-chunk j."""
    idx = []
    for j in range(4):
        for base in (0, H, 2 * H, 3 * H):           # i, f, g, o
            idx.extend(range(base + j * 128, base + (j + 1) * 128))
    return np.array(idx)


def _pack_host(messages, W_ih, W_hh, b_ih, b_hh, W_fc, b_fc):
    perm = _gate_perm()
    W_hh_p = W_hh[perm]                              # [2048, 512]
    whh = np.zeros((128, KT * MT * 128), np.float32)
    for k in range(KT):
        for m in range(MT):
            t_ = W_hh_p[m * 128:(m + 1) * 128, k * 128:(k + 1) * 128].T
            whh[:, (k * MT + m) * 128:(k * MT + m + 1) * 128] = t_
    whh = whh.astype(ml_dtypes.bfloat16)

    W_ih_p = W_ih[perm]                              # [2048, 25]
    bias_p = (b_ih + b_hh)[perm]                     # [2048]
    wih_aug = np.zeros((26, 4 * H), np.float32)
    wih_aug[:25] = W_ih_p.T
    wih_aug[25] = bias_p
    wih = np.zeros((128, 4 * H), np.float32)
    for r in range(4):
        wih[r * 32:r * 32 + 26] = wih_aug
    wih = wih.astype(ml_dtypes.bfloat16)

    wfc = np.zeros((128, KT * NCLS), np.float32)
    W_fc_T = W_fc.T                                  # [512, 100]
    for k in range(KT):
        wfc[:, k * NCLS:(k + 1) * NCLS] = W_fc_T[k * 128:(k + 1) * 128]
    wfc = wfc.astype(ml_dtypes.bfloat16)

    bfc = b_fc.astype(np.float32).reshape(NCLS, 1)

    in_maps = []
    for c in range(CORES):
        shard = messages[c * BS:(c + 1) * BS]        # [BS, T, V]
        xT = np.ascontiguousarray(shard.transpose(2, 1, 0))  # [V, T, BS]
        x_rep = np.zeros((128, T, BS), np.float32)
        for r in range(4):
            x_rep[r * 32:r * 32 + 25] = xT
            x_rep[r * 32 + 25] = 1.0
        x_rep = x_rep.reshape(128, T * BS).astype(ml_dtypes.bfloat16)
        in_maps.append({"x_rep": x_rep, "whh": whh, "wih": wih,
                        "wfc": wfc, "bfc": bfc})
    return in_maps


def _build(reps=1):
    nc = bacc.Bacc("TRN2", target_bir_lowering=False, debug=False)

    x_dram = nc.dram_tensor("x_rep", [128, T * BS], BF16,
                            kind="ExternalInput").ap()
    whh_dram = nc.dram_tensor("whh", [128, KT * MT * 128], BF16,
                              kind="ExternalInput").ap()
    wih_dram = nc.dram_tensor("wih", [128, 4 * H], BF16,
                              kind="ExternalInput").ap()
    wfc_dram = nc.dram_tensor("wfc", [128, KT * NCLS], BF16,
                              kind="ExternalInput").ap()
    bfc_dram = nc.dram_tensor("bfc", [NCLS, 1], F32,
                              kind="ExternalInput").ap()
    out_dram = nc.dram_tensor("out", [NCLS, BS], F32,
                              kind="ExternalOutput").ap()

    with tile.TileContext(nc) as tc:
        with (
            tc.tile_pool(name="const", bufs=1) as cpool,
            tc.tile_pool(name="xbuf", bufs=1) as xpool,
            tc.tile_pool(name="state", bufs=1) as spool,
            tc.tile_pool(name="psum", bufs=1, space="PSUM") as ppool,
            tc.tile_pool(name="work", bufs=4) as wpool,
        ):
            whh_sb = cpool.tile([128, KT * MT * 128], BF16)
            wih_sb = cpool.tile([128, 4 * H], BF16)
            wfc_sb = cpool.tile([128, KT * NCLS], BF16)
            bfc_sb = cpool.tile([NCLS, 1], F32)
            x_sb = xpool.tile([128, T * BS], BF16)
            h_sb = [spool.tile([128, KT * BS], BF16, name=f"h{p}")
                    for p in range(2)]
            c_sb = spool.tile([128, KT * BS], BF16)

            # Prologue DMAs, ordered so step 0's dependencies land first:
            # wih + x chunk 0 (X phase), then whh k-chunk by k-chunk.
            nc.sync.dma_start(wih_sb[:], wih_dram[:])
            xc = T * BS // 8
            nc.sync.dma_start(x_sb[:, 0:xc], x_dram[:, 0:xc])
            wc = MT * 128
            for k in range(KT):
                nc.sync.dma_start(whh_sb[:, k * wc:(k + 1) * wc],
                                  whh_dram[:, k * wc:(k + 1) * wc])
            for i in range(1, 8):
                nc.sync.dma_start(x_sb[:, i * xc:(i + 1) * xc],
                                  x_dram[:, i * xc:(i + 1) * xc])
            nc.sync.dma_start(wfc_sb[:], wfc_dram[:])
            nc.sync.dma_start(bfc_sb[:], bfc_dram[:])

            gb = []
            for b_ in range(NB):
                t_ = ppool.tile([128, 512], F32, name=f"gbank{b_}")
                gb.append(t_)

            for _rep in range(reps):
              for t in range(T):
                xs = x_sb[:, t * BS:(t + 1) * BS]
                hr = h_sb[t % 2]           # h(t-1)
                hw = h_sb[(t + 1) % 2]     # h(t)
                # X phase: depends only on x, emitted first so the PE has
                # deferred work while the previous step's h chunks finish.
                for m in range(MT):
                    r = m % 4
                    nc.tensor.matmul(
                        gb[m // 2][:, (m % 2) * BS:(m % 2 + 1) * BS],
                        wih_sb[r * 32:r * 32 + 26,
                               m * 128:(m + 1) * 128],
                        xs[r * 32:r * 32 + 26, :],
                        start=(m % 2 == 0),
                        stop=(t == 0 and m % 2 == 1),
                        tile_position=(r * 32, 0),
                    )
                for j in range(4):
                    if t > 0:
                        # k ascending: the last-produced h chunk (k=3) is
                        # consumed as late as possible within the group.
                        for k in range(KT):
                            for q in range(4):
                                m = 4 * j + q
                                nc.tensor.matmul(
                                    gb[m // 2][:, (m % 2) * BS:
                                               (m % 2 + 1) * BS],
                                    whh_sb[:, (k * MT + m) * 128:
                                           (k * MT + m + 1) * 128],
                                    hr[:, k * BS:(k + 1) * BS],
                                    start=False,
                                    stop=(k == KT - 1 and q % 2 == 1),
                                )
                    if_t = wpool.tile([128, 512], BF16, tag="if")
                    g_t = wpool.tile([128, BS], BF16, tag="g")
                    o_t = wpool.tile([128, BS], BF16, tag="o")
                    tc_t = wpool.tile([128, BS], BF16, tag="tc")
                    cj = c_sb[:, j * BS:(j + 1) * BS]
                    nc.scalar.activation(if_t[:], gb[2 * j][:], AF.Sigmoid)
                    nc.scalar.activation(g_t[:], gb[2 * j + 1][:, 0:BS],
                                         AF.Tanh)
                    nc.scalar.activation(o_t[:], gb[2 * j + 1][:, BS:2 * BS],
                                         AF.Sigmoid)
                    if t == 0:
                        # c(0) = i*g  (f*c term is zero; nothing to reset)
                        nc.vector.tensor_mul(cj, if_t[:, 0:BS], g_t[:])
                    else:
                        ig_t = wpool.tile([128, BS], BF16, tag="ig")
                        fc_t = wpool.tile([128, BS], BF16, tag="fc")
                        nc.vector.tensor_mul(ig_t[:], if_t[:, 0:BS], g_t[:])
                        nc.gpsimd.tensor_mul(fc_t[:], if_t[:, BS:2 * BS], cj)
                        nc.vector.tensor_add(cj, ig_t[:], fc_t[:])
                    nc.scalar.activation(tc_t[:], cj, AF.Tanh)
                    nc.vector.tensor_mul(hw[:, j * BS:(j + 1) * BS],
                                         o_t[:], tc_t[:])

            # Final h lives in h_sb[T % 2] == h_sb[0] (T even).
            for k in range(KT):
                nc.tensor.matmul(
                    gb[0][0:NCLS, 0:BS],
                    wfc_sb[:, k * NCLS:(k + 1) * NCLS],
                    h_sb[T % 2][:, k * BS:(k + 1) * BS],
                    start=(k == 0), stop=(k == KT - 1),
                )
            out_sb = cpool.tile([NCLS, BS], F32)
            nc.scalar.activation(out_sb[:], gb[0][0:NCLS, 0:BS],
                                 AF.Identity, bias=bfc_sb[:])
            nc.sync.dma_start(out_dram[:], out_sb[:])

    nc.compile()
    return nc


_NC_CACHE = None


def kernel(messages, W_ih, W_hh, b_ih, b_hh, W_fc, b_fc):
    """Full-input entry point: shard, run on 8 NeuronCores, gather."""
    global _NC_CACHE
    messages = np.asarray(messages, np.float32)
    W_ih = np.asarray(W_ih, np.float32)
    W_hh = np.asarray(W_hh, np.float32)
    b_ih = np.asarray(b_ih, np.float32)
    b_hh = np.asarray(b_hh, np.float32)
    W_fc = np.asarray(W_fc, np.float32)
    b_fc = np.asarray(b_fc, np.float32)

    in_maps = _pack_host(messages, W_ih, W_hh, b_ih, b_hh, W_fc, b_fc)
    if _NC_CACHE is None:
        _NC_CACHE = _build()
    res = run_bass_kernel_spmd(_NC_CACHE, in_maps, list(range(CORES)))
    outs = [np.ascontiguousarray(np.asarray(res.results[c]["out"]).T)
            for c in range(CORES)]
    return np.concatenate(outs, axis=0).astype(np.float32)


# revision 6
# speedup vs baseline: 12.8464x; 12.8464x over previous
"""Trainium2 Bass kernel for nn_DiagnosticRNN (LSTM B=2048,T=128,V=25,H=512
-> FC 100), 8-way batch-data-parallel across NeuronCores.

Strategy
--------
Data-parallel over batch: each of the 8 cores runs the full T=128 LSTM
recurrence on BS=256 batch rows with all weights replicated (per the
sharding hint). Everything is fused on-chip; the naive approach's
[B,T,4H] x-projection (2.1 GB of DRAM traffic) is never materialized.

Per-core per-timestep (all matmul operands bf16, fp32 PSUM accumulate):
  gates[4H, BS] = W_hh_perm @ h_{t-1} + W_ihaug_perm @ [x_t; 1]
    * 16 M-tiles (permuted gate order: m = 4j+q, q in (i,f,g,o), j the
      h-chunk), N=BS=256, PSUM bank b holds M-tile pair (2b, 2b+1).
    * x-term: thin K=26 matmuls (V=25 + a ones row folding b_ih+b_hh),
      4-way row-tiled via tile_position (x replicated at partition
      offsets 0/32/64/96); even m start=True then odd m start=False so
      each PSUM bank's has_written clear happens exactly once per step.
    * W_hh term: 64 MMs, k-outer so step t+1's k-th sweep only needs
      h-chunk k -> deep cross-step pipelining with ACT/DVE.
  ScalarE: sigmoid([i_j f_j] fused 512 cols), tanh(g_j), sigmoid(o_j),
  tanh(c_j), all PSUM->SBUF, bf16 outputs.
  VectorE/GpSimd: ig = i*g, fc = f*c (GpSimd), c = ig+fc, h = o*tanh(c),
  bf16 for DVE 2x mode.
  FC epilogue: out[100, BS] = W_fc @ h (+b_fc via ACT Identity bias);
  host transposes to [BS, 100].

Host side packs/permutes/casts the weights and pre-transposes messages
into x_rep [128, T*BS] bf16 (4 replicas of [V+1, t, b] at partition
offsets 0/32/64/96). All numerics on device; bf16 operand rounding gives
~5e-3 scale-relative absmax vs the fp32 reference.
"""

import numpy as np
import ml_dtypes

import concourse.bacc as bacc
import concourse.mybir as mybir
import concourse.tile as tile
from concourse.bass_utils import run_bass_kernel_spmd

F32 = mybir.dt.float32
BF16 = mybir.dt.bfloat16
AF = mybir.ActivationFunctionType

B, T, V = 2048, 128, 25
H = 512
NCLS = 100
CORES = 8
BS = B // CORES          # 256 batch rows per core
KT = H // 128            # 4 k-tiles (h chunks)
MT = (4 * H) // 128      # 16 m-tiles
NB = 8                   # psum banks


def _gate_perm():
    """Permutation of the 4H gate dim: m-tile m=4j+q -> gate q, h-chunk j."""
    idx = []
    for j in range(4):
        for base in (0, H, 2 * H, 3 * H):           # i, f, g, o
            idx.extend(range(base + j * 128, base + (j + 1) * 128))
    return np.array(idx)


def _pack_host(messages, W_ih, W_hh, b_ih, b_hh, W_fc, b_fc):
    perm = _gate_perm()
    W_hh_p = W_hh[perm]                              # [2048, 512]
    whh = np.zeros((128, KT * MT * 128), np.float32)
    for k in range(KT):
        for m in range(MT):
            t_ = W_hh_p[m * 128:(m + 1) * 128, k * 128:(k + 1) * 128].T
            whh[:, (k * MT + m) * 128:(k * MT + m + 1) * 128] = t_
    whh = whh.astype(ml_dtypes.bfloat16)

    W_ih_p = W_ih[perm]                              # [2048, 25]
    bias_p = (b_ih + b_hh)[perm]                     # [2048]
    wih_aug = np.zeros((26, 4 * H), np.float32)
    wih_aug[:25] = W_ih_p.T
    wih_aug[25] = bias_p
    wih = np.zeros((128, 4 * H), np.float32)
    for r in range(4):
        wih[r * 32:r * 32 + 26] = wih_aug
    wih = wih.astype(ml_dtypes.bfloat16)

    wfc = np.zeros((128, KT * NCLS), np.float32)
    W_fc_T = W_fc.T                                  # [512, 100]
    for k in range(KT):
        wfc[:, k * NCLS:(k + 1) * NCLS] = W_fc_T[k * 128:(k + 1) * 128]
    wfc = wfc.astype(ml_dtypes.bfloat16)

    bfc = b_fc.astype(np.float32).reshape(NCLS, 1)

    in_maps = []
    for c in range(CORES):
        shard = messages[c * BS:(c + 1) * BS]        # [BS, T, V]
        xT = np.ascontiguousarray(shard.transpose(2, 1, 0))  # [V, T, BS]
        x_rep = np.zeros((128, T, BS), np.float32)
        for r in range(4):
            x_rep[r * 32:r * 32 + 25] = xT
            x_rep[r * 32 + 25] = 1.0
        x_rep = x_rep.reshape(128, T * BS).astype(ml_dtypes.bfloat16)
        in_maps.append({"x_rep": x_rep, "whh": whh, "wih": wih,
                        "wfc": wfc, "bfc": bfc})
    return in_maps


def _build(reps=1):
    nc = bacc.Bacc("TRN2", target_bir_lowering=False, debug=False)

    x_dram = nc.dram_tensor("x_rep", [128, T * BS], BF16,
                            kind="ExternalInput").ap()
    whh_dram = nc.dram_tensor("whh", [128, KT * MT * 128], BF16,
                              kind="ExternalInput").ap()
    wih_dram = nc.dram_tensor("wih", [128, 4 * H], BF16,
                              kind="ExternalInput").ap()
    wfc_dram = nc.dram_tensor("wfc", [128, KT * NCLS], BF16,
                              kind="ExternalInput").ap()
    bfc_dram = nc.dram_tensor("bfc", [NCLS, 1], F32,
                              kind="ExternalInput").ap()
    out_dram = nc.dram_tensor("out", [NCLS, BS], F32,
                              kind="ExternalOutput").ap()

    with tile.TileContext(nc) as tc:
        with (
            tc.tile_pool(name="const", bufs=1) as cpool,
            tc.tile_pool(name="xbuf", bufs=1) as xpool,
            tc.tile_pool(name="state", bufs=1) as spool,
            tc.tile_pool(name="psum", bufs=1, space="PSUM") as ppool,
            tc.tile_pool(name="work", bufs=3) as wpool,
        ):
            whh_sb = cpool.tile([128, KT * MT * 128], BF16)
            wih_sb = cpool.tile([128, 4 * H], BF16)
            wfc_sb = cpool.tile([128, KT * NCLS], BF16)
            bfc_sb = cpool.tile([NCLS, 1], F32)
            x_sb = xpool.tile([128, T * BS], BF16)
            h_sb = spool.tile([128, KT * BS], BF16)
            c_sb = spool.tile([128, KT * BS], BF16)

            nc.sync.dma_start(whh_sb[:], whh_dram[:])
            nc.sync.dma_start(wih_sb[:], wih_dram[:])
            nc.sync.dma_start(wfc_sb[:], wfc_dram[:])
            nc.sync.dma_start(bfc_sb[:], bfc_dram[:])
            xc = T * BS // 8
            for i in range(8):
                nc.sync.dma_start(x_sb[:, i * xc:(i + 1) * xc],
                                  x_dram[:, i * xc:(i + 1) * xc])

            gb = []
            for b_ in range(NB):
                t_ = ppool.tile([128, 512], F32, name=f"gbank{b_}")
                gb.append(t_)

            for _rep in range(reps):
              nc.vector.memset(h_sb[:], 0.0)
              nc.vector.memset(c_sb[:], 0.0)
              for t in range(T):
                xs = x_sb[:, t * BS:(t + 1) * BS]
                for phase in range(2):
                    for m in range(phase, MT, 2):
                        r = (m // 2) % 4
                        nc.tensor.matmul(
                            gb[m // 2][:, (m % 2) * BS:(m % 2 + 1) * BS],
                            wih_sb[r * 32:r * 32 + 26,
                                   m * 128:(m + 1) * 128],
                            xs[r * 32:r * 32 + 26, :],
                            start=(phase == 0), stop=False,
                            tile_position=(r * 32, 0),
                        )
                for k in range(KT):
                    for m in range(MT):
                        nc.tensor.matmul(
                            gb[m // 2][:, (m % 2) * BS:(m % 2 + 1) * BS],
                            whh_sb[:, (k * MT + m) * 128:
                                   (k * MT + m + 1) * 128],
                            h_sb[:, k * BS:(k + 1) * BS],
                            start=False,
                            stop=(k == KT - 1 and m % 2 == 1),
                        )
                for j in range(4):
                    if_t = wpool.tile([128, 512], BF16, tag="if")
                    g_t = wpool.tile([128, BS], BF16, tag="g")
                    o_t = wpool.tile([128, BS], BF16, tag="o")
                    ig_t = wpool.tile([128, BS], BF16, tag="ig")
                    fc_t = wpool.tile([128, BS], BF16, tag="fc")
                    tc_t = wpool.tile([128, BS], BF16, tag="tc")
                    nc.scalar.activation(if_t[:], gb[2 * j][:], AF.Sigmoid)
                    nc.scalar.activation(g_t[:], gb[2 * j + 1][:, 0:BS],
                                         AF.Tanh)
                    nc.scalar.activation(o_t[:], gb[2 * j + 1][:, BS:2 * BS],
                                         AF.Sigmoid)
                    cj = c_sb[:, j * BS:(j + 1) * BS]
                    nc.vector.tensor_mul(ig_t[:], if_t[:, 0:BS], g_t[:])
                    nc.gpsimd.tensor_mul(fc_t[:], if_t[:, BS:2 * BS], cj)
                    nc.vector.tensor_add(cj, ig_t[:], fc_t[:])
                    nc.scalar.activation(tc_t[:], cj, AF.Tanh)
                    nc.vector.tensor_mul(h_sb[:, j * BS:(j + 1) * BS],
                                         o_t[:], tc_t[:])

            for k in range(KT):
                nc.tensor.matmul(
                    gb[0][0:NCLS, 0:BS],
                    wfc_sb[:, k * NCLS:(k + 1) * NCLS],
                    h_sb[:, k * BS:(k + 1) * BS],
                    start=(k == 0), stop=(k == KT - 1),
                )
            out_sb = cpool.tile([NCLS, BS], F32)
            nc.scalar.activation(out_sb[:], gb[0][0:NCLS, 0:BS],
                                 AF.Identity, bias=bfc_sb[:])
            nc.sync.dma_start(out_dram[:], out_sb[:])

    nc.compile()
    return nc


_NC_CACHE = None


def kernel(messages, W_ih, W_hh, b_ih, b_hh, W_fc, b_fc):
    """Full-input entry point: shard, run on 8 NeuronCores, gather."""
    global _NC_CACHE
    messages = np.asarray(messages, np.float32)
    W_ih = np.asarray(W_ih, np.float32)
    W_hh = np.asarray(W_hh, np.float32)
    b_ih = np.asarray(b_ih, np.float32)
    b_hh = np.asarray(b_hh, np.float32)
    W_fc = np.asarray(W_fc, np.float32)
    b_fc = np.asarray(b_fc, np.float32)

    in_maps = _pack_host(messages, W_ih, W_hh, b_ih, b_hh, W_fc, b_fc)
    if _NC_CACHE is None:
        _NC_CACHE = _build()
    res = run_bass_kernel_spmd(_NC_CACHE, in_maps, list(range(CORES)))
    outs = [np.ascontiguousarray(np.asarray(res.results[c]["out"]).T)
            for c in range(CORES)]
    return np.concatenate(outs, axis=0).astype(np.float32)

